# revision 1
# baseline (speedup 1.0000x reference)
"""Trainium2 Bass kernel for nn_PolyAttention (16-head polynomial causal attention).

Reference math (fp32):
    q = x @ Wq.T; k = x @ Wk.T; v = x @ Wv.T        (per-head dim 128, 16 heads)
    q, k = rope(q), rope(k)                          (LRPE type-1, base 10000)
    s = (q . k)^4, causal-masked, row-normalized by max(sum, 1e-6)
    out = (s @ v normalized) @ Wo.T

Sharding: 8 cores = batch(2) x head-group(4 heads each).  Each core computes its
(b, head-group) shard end-to-end plus the Wo partial projection; the host sums
the 4 partials per batch element.

Device layout notes (per core):
  xt  [2048,2048]  x[b].T                (d on partitions, n on free dim)
  wq/wk/wv [2048, 512]   W[g_rows].T     (d x local-head-dims)
  wo  [512, 2048]        Wo[:, g_cols].T (local-c x d_out)
  qT/kT SBUF [128, 4*2048]  per-head transposed activations (dh x n), roped
  vS  SBUF [128, 16*512]    v blocks, kb-major (key-in-block x (kb, h, dh))
  scores are built transposed: sT [keys, queries] so that AV yields outT [dh, q]
  directly and the Wo matmul needs no transposes anywhere.
"""

import os
import sys

import numpy as np

if "/opt/trn_rl_repo" not in sys.path:
    sys.path.insert(0, "/opt/trn_rl_repo")

# ---------------------------------------------------------------- constants
B = 2
N = 2048
D = 2048
NH = 16
DH = 128
NHL = 4          # heads per core
HL = NHL * DH    # 512 local head dims
POLY = 4
EPS = 1e-6
LRPE_BASE = 10000.0

CH = 256         # projection n-chunk (columns of xT per step)
QB = 512         # query block
KB = 128         # key block

USE_F32R = os.environ.get("POLY_F32R", "1") == "1"


# ---------------------------------------------------------------- builder
def build_module(n=N, use_f32r=USE_F32R):
    import concourse.bacc as bacc
    import concourse.mybir as mybir
    import concourse.tile as tile

    f32 = mybir.dt.float32
    f32r = mybir.dt.float32r
    AF = mybir.ActivationFunctionType

    nc = bacc.Bacc(
        "TRN2",
        target_bir_lowering=False,
        debug=False,
        enable_asserts=False,
        num_devices=8,
    )
    mdt = f32r if use_f32r else f32

    xt_d = nc.dram_tensor("xt", [D, n], mdt, kind="ExternalInput").ap()
    wq_d = nc.dram_tensor("wq", [D, HL], mdt, kind="ExternalInput").ap()
    wk_d = nc.dram_tensor("wk", [D, HL], mdt, kind="ExternalInput").ap()
    wv_d = nc.dram_tensor("wv", [D, HL], mdt, kind="ExternalInput").ap()
    wo_d = nc.dram_tensor("wo", [HL, D], mdt, kind="ExternalInput").ap()
    cs_d = nc.dram_tensor("cs", [DH, n], f32, kind="ExternalInput").ap()
    sn_d = nc.dram_tensor("sn", [DH, n], f32, kind="ExternalInput").ap()
    mk_d = nc.dram_tensor("msk", [KB, 3 * KB], f32, kind="ExternalInput").ap()
    out_d = nc.dram_tensor("out", [n, D], f32, kind="ExternalOutput").ap()

    ndb = D // 128          # 16 d-blocks (contraction tiles)
    nch = n // CH           # projection chunks
    nqb = n // QB           # query blocks
    nkb_tot = n // KB       # key blocks

    def mm(out, lhsT, rhs, start, stop):
        nc.tensor.matmul(out, lhsT, rhs, start=start, stop=stop)

    with tile.TileContext(nc) as tc:
        from contextlib import ExitStack

        with ExitStack() as ctx:
            persist = ctx.enter_context(tc.tile_pool(name="persist", bufs=1))
            qT = persist.tile([128, NHL * n], mdt, tag="qT", name="qT")
            kT = persist.tile([128, NHL * n], mdt, tag="kT", name="kT")
            vS = persist.tile([128, nkb_tot * HL], mdt, tag="vS", name="vS")
            ones = persist.tile([128, 1], mdt, tag="ones", name="ones")
            nc.vector.memset(ones[:].bitcast(f32), 1.0)

            # ---------------- pass A: q,k projections + rope ----------------
            with tc.tile_pool(name="pA_w", bufs=1) as wpool, \
                 tc.tile_pool(name="pA_x", bufs=2) as xpool, \
                 tc.tile_pool(name="pA_t", bufs=3) as tpool, \
                 tc.tile_pool(name="pA_ps", bufs=6, space="PSUM") as pspool:
                wq_t = [wpool.tile([128, HL], mdt, tag=f"wq{i}", name=f"wq{i}") for i in range(ndb)]
                wk_t = [wpool.tile([128, HL], mdt, tag=f"wk{i}", name=f"wk{i}") for i in range(ndb)]

                for c in range(nch):
                    c0 = c * CH
                    xt_c = [xpool.tile([128, CH], mdt, tag=f"xt{i}", name=f"xt{i}") for i in range(ndb)]
                    cs_c = xpool.tile([128, CH], f32, tag="cs", name="cs")
                    sn_c = xpool.tile([128, CH], f32, tag="sn", name="sn")
                    nc.sync.dma_start(cs_c[:], cs_d[:, c0:c0 + CH])
                    nc.sync.dma_start(sn_c[:], sn_d[:, c0:c0 + CH])
                    # interleave weight loads with the first x chunk so the
                    # first accumulation chain starts after ~3 tiles, not 10MB
                    for i in range(ndb):
                        nc.sync.dma_start(xt_c[i][:], xt_d[i * 128:(i + 1) * 128, c0:c0 + CH])
                        if c == 0:
                            nc.sync.dma_start(wq_t[i][:], wq_d[i * 128:(i + 1) * 128, :])
                            nc.sync.dma_start(wk_t[i][:], wk_d[i * 128:(i + 1) * 128, :])

                    for h in range(NHL):
                        for w_t, dstT in ((wq_t, qT), (wk_t, kT)):
                            ps = pspool.tile([128, CH], f32, tag="ps", name="ps")
                            for i in range(ndb):
                                mm(ps[:], w_t[i][:, h * 128:(h + 1) * 128], xt_c[i][:],
                                   start=(i == 0), stop=(i == ndb - 1))
                            # rope: dst = ps*CS + swap(ps)*SN
                            dst = dstT[:, h * n + c0: h * n + c0 + CH]
                            swp = tpool.tile([128, CH], f32, tag="swp", name="swp")
                            nc.scalar.copy(swp[0:64, :], ps[64:128, :])
                            nc.scalar.copy(swp[64:128, :], ps[0:64, :])
                            m1 = tpool.tile([128, CH], f32, tag="m1", name="m1")
                            nc.vector.tensor_mul(m1[:], ps[:], cs_c[:])
                            m2 = tpool.tile([128, CH], f32, tag="m2", name="m2")
                            nc.gpsimd.tensor_mul(m2[:], swp[:], sn_c[:])
                            nc.vector.tensor_add(dst, m1[:], m2[:])

            # ---------------- pass B: v projection ----------------
            with tc.tile_pool(name="pB_w", bufs=1) as wpool, \
                 tc.tile_pool(name="pB_x", bufs=2) as xpool, \
                 tc.tile_pool(name="pB_ps", bufs=4, space="PSUM") as pspool:
                wv_t = [wpool.tile([128, HL], mdt, tag=f"wv{i}", name=f"wv{i}") for i in range(ndb)]
                for c in range(nch):
                    c0 = c * CH
                    xt_c = [xpool.tile([128, CH], mdt, tag=f"xt{i}", name=f"xt{i}") for i in range(ndb)]
                    for i in range(ndb):
                        nc.sync.dma_start(xt_c[i][:], xt_d[i * 128:(i + 1) * 128, c0:c0 + CH])
                        if c == 0:
                            nc.sync.dma_start(wv_t[i][:], wv_d[i * 128:(i + 1) * 128, :])
                    for t2 in range(CH // 128):
                        kb = c * (CH // 128) + t2
                        ps = pspool.tile([128, HL], f32, tag="psv", name="psv")
                        for i in range(ndb):
                            mm(ps[:], xt_c[i][:, t2 * 128:(t2 + 1) * 128], wv_t[i][:],
                               start=(i == 0), stop=(i == ndb - 1))
                        nc.scalar.copy(vS[:, kb * HL:(kb + 1) * HL], ps[:])

            # ---------------- pass C: attention + Wo ----------------
            with tc.tile_pool(name="pC_w", bufs=1) as wpool, \
                 tc.tile_pool(name="pC_sb", bufs=4) as spool, \
                 tc.tile_pool(name="pC_on", bufs=2) as onpool, \
                 tc.tile_pool(name="pC_fo", bufs=2) as fopool, \
                 tc.tile_pool(name="pC_ps", bufs=4, space="PSUM") as psS, \
                 tc.tile_pool(name="pC_po", bufs=2, space="PSUM") as psO, \
                 tc.tile_pool(name="pC_pd", bufs=1, space="PSUM") as psD, \
                 tc.tile_pool(name="pC_pf", bufs=1, space="PSUM") as psF:
                wo_t = [wpool.tile([128, D], mdt, tag=f"wo{i}", name=f"wo{i}") for i in range(NHL)]
                mk = wpool.tile([128, 3 * KB], f32, tag="mk", name="mk")
                nc.sync.dma_start(mk[:], mk_d[:, :])
                wo_emitted = False

                for qb in range(nqb):
                    onrm = [onpool.tile([128, QB], mdt, tag=f"onrm{h}", name=f"onrm{h}") for h in range(NHL)]
                    for h in range(NHL):
                        nkb = (qb + 1) * (QB // KB)
                        pso = psO.tile([128, QB], f32, tag="pso", name="pso")
                        psd = psD.tile([1, QB], f32, tag="psd", name="psd")
                        for kb in range(nkb):
                            rel = kb - qb * (QB // KB)
                            # band blocks: only queries >= cr can attend to this
                            # key block; skip the dead columns entirely
                            cr = 0 if rel < 0 else min(KB * rel, 2 * KB)
                            w = QB - cr
                            pss = psS.tile([128, QB], f32, tag="pss", name="pss")
                            mm(pss[:, cr:], kT[:, h * n + kb * KB: h * n + (kb + 1) * KB],
                               qT[:, h * n + qb * QB + cr: h * n + (qb + 1) * QB],
                               start=True, stop=True)
                            s2 = spool.tile([128, QB], f32, tag="s2", name="s2")
                            nc.scalar.activation(s2[:, cr:], pss[:, cr:], AF.Square)
                            if rel >= 0:
                                if rel < 3:
                                    nc.gpsimd.tensor_mul(s2[:, KB * rel:KB * (rel + 1)],
                                                         s2[:, KB * rel:KB * (rel + 1)],
                                                         mk[:, 0:KB])
                                else:
                                    nc.gpsimd.tensor_mul(s2[:, 2 * KB:], s2[:, 2 * KB:],
                                                         mk[:, KB:3 * KB])
                            s4 = spool.tile([128, QB], mdt, tag="s4", name="s4")
                            if kb % 4 == 2:
                                nc.gpsimd.tensor_mul(s4[:, cr:], s2[:, cr:], s2[:, cr:])
                            else:
                                nc.vector.tensor_mul(s4[:, cr:], s2[:, cr:], s2[:, cr:])
                            mm(pso[:, cr:], vS[:, kb * HL + h * 128: kb * HL + (h + 1) * 128],
                               s4[:, cr:], start=(kb == 0), stop=(kb == nkb - 1))
                            mm(psd[0:1, cr:], ones[:, 0:1], s4[:, cr:],
                               start=(kb == 0), stop=(kb == nkb - 1))
                        if qb == 0 and h == 0 and not wo_emitted:
                            # prefetch Wo during the first head's attention
                            wo_emitted = True
                            for i in range(NHL):
                                nc.sync.dma_start(wo_t[i][:], wo_d[i * 128:(i + 1) * 128, :])
                        rec = spool.tile([1, QB], f32, tag="rec", name="rec")
                        nc.vector.tensor_scalar_max(rec[:], psd[0:1, :], EPS)
                        rec2 = spool.tile([1, QB], f32, tag="rec2", name="rec2")
                        nc.vector.reciprocal(rec2[:], rec[:])
                        rbc = spool.tile([128, QB], f32, tag="rbc", name="rbc")
                        nc.gpsimd.partition_broadcast(rbc[:], rec2[:])
                        nc.vector.tensor_mul(onrm[h][:], pso[:], rbc[:])
                    # Wo projection for this query block
                    for qt in range(QB // 128):
                        fout = fopool.tile([128, D], f32, tag="fout", name="fout")
                        for jc in range(D // 512):
                            psf = psF.tile([128, 512], f32, tag="psf", name="psf")
                            for h in range(NHL):
                                mm(psf[:], onrm[h][:, qt * 128:(qt + 1) * 128],
                                   wo_t[h][:, jc * 512:(jc + 1) * 512],
                                   start=(h == 0), stop=(h == NHL - 1))
                            if jc % 2 == 0:
                                nc.scalar.copy(fout[:, jc * 512:(jc + 1) * 512], psf[:])
                            else:
                                nc.vector.tensor_copy(fout[:, jc * 512:(jc + 1) * 512], psf[:])
                        r0 = qb * QB + qt * 128
                        nc.sync.dma_start(out_d[r0:r0 + 128, :], fout[:])

    nc.compile()
    return nc


# ---------------------------------------------------------------- host prep
def _rope_tables(n):
    half = DH // 2
    theta = LRPE_BASE ** (-np.arange(half, dtype=np.float64) * 2.0 / DH)
    pos = np.arange(n, dtype=np.float64)
    ang = np.outer(pos, theta)                       # [n, 64]
    cos = np.cos(ang).T.astype(np.float32)           # [64, n]
    sin = np.sin(ang).T.astype(np.float32)
    cs = np.concatenate([cos, cos], axis=0)          # [128, n]
    sn = np.concatenate([-sin, sin], axis=0)
    return np.ascontiguousarray(cs), np.ascontiguousarray(sn)


def _masks():
    # cols 0:128   = tri mask (kp <= j), applied to the diagonal 128-col strip
    #                of rel-0/1/2 band blocks
    # cols 128:384 = rel-3 mask over its 256 computed cols (kp <= j - 128)
    out = np.zeros((KB, 3 * KB), dtype=np.float32)
    kp = np.arange(KB)[:, None]
    j1 = np.arange(KB)[None, :]
    j2 = np.arange(2 * KB)[None, :]
    out[:, :KB] = (kp <= j1).astype(np.float32)
    out[:, KB:] = (kp <= j2 - KB).astype(np.float32)
    return out


def make_in_maps(x, Wq, Wk, Wv, Wo, n=N):
    cs, sn = _rope_tables(n)
    mk = _masks()
    xts = [np.ascontiguousarray(x[b].T) for b in range(x.shape[0])]
    in_maps = []
    for core in range(8):
        b, g = core // 4, core % 4
        rows = slice(g * HL, (g + 1) * HL)
        in_maps.append({
            "xt": xts[b],
            "wq": np.ascontiguousarray(Wq[rows, :].T),
            "wk": np.ascontiguousarray(Wk[rows, :].T),
            "wv": np.ascontiguousarray(Wv[rows, :].T),
            "wo": np.ascontiguousarray(Wo[:, rows].T),
            "cs": cs,
            "sn": sn,
            "msk": mk,
        })
    return in_maps


_NC_CACHE = {}


def _get_nc(n=N, use_f32r=USE_F32R):
    key = (n, use_f32r)
    if key not in _NC_CACHE:
        _NC_CACHE[key] = build_module(n, use_f32r)
    return _NC_CACHE[key]


def run(x, Wq, Wk, Wv, Wo, trace=False, **kw):
    from concourse.bass_utils import run_bass_kernel_spmd

    x = np.asarray(x, dtype=np.float32)
    nc = _get_nc(x.shape[1])
    in_maps = make_in_maps(x, Wq, Wk, Wv, Wo, n=x.shape[1])
    res = run_bass_kernel_spmd(nc, in_maps, core_ids=list(range(8)), trace=trace, **kw)
    b0 = res.results[0]["out"] + res.results[1]["out"] + res.results[2]["out"] + res.results[3]["out"]
    b1 = res.results[4]["out"] + res.results[5]["out"] + res.results[6]["out"] + res.results[7]["out"]
    out = np.stack([b0, b1]).astype(np.float32)
    return out, res


def kernel(x, Wq, Wk, Wv, Wo):
    out, _ = run(
        np.asarray(x, np.float32),
        np.asarray(Wq, np.float32),
        np.asarray(Wk, np.float32),
        np.asarray(Wv, np.float32),
        np.asarray(Wo, np.float32),
    )
    return out



# revision 2
# speedup vs baseline: 1.0910x; 1.0910x over previous
"""Trainium2 Bass kernel for nn_PolyAttention (16-head polynomial causal attention).

Reference math (fp32):
    q = x @ Wq.T; k = x @ Wk.T; v = x @ Wv.T        (per-head dim 128, 16 heads)
    q, k = rope(q), rope(k)                          (LRPE type-1, base 10000)
    s = (q . k)^4, causal-masked, row-normalized by max(sum, 1e-6)
    out = (s @ v normalized) @ Wo.T

Sharding: 8 cores = batch(2) x head-group(4 heads each).  Each core computes its
(b, head-group) shard end-to-end plus the Wo partial projection; the host sums
the 4 partials per batch element.

v2 design (vs v1):
  - all matmul operands bf16 (host-converted); fp32 PSUM accumulate; fp16 out.
    Numerics sim: rel_fro ~7e-3 (gate 2e-2).  bf16 halves DMA + SBUF and
    enables FWL fast weight loads.
  - single merged projection pass: x loaded once, q/k/v computed per n-chunk.
  - host relayouts inputs so every chunk/weight load is ONE dma_start with
    16KB contiguous per partition line.
  - attention: scores built transposed [keys, queries]; 2-block software
    pipeline (score chain runs 2 blocks ahead of the AV chain) so the PE
    never waits on the scalar/vector square/quartic pipeline.
  - denominator off the PE: DVE accumulates s4 blocks into sAcc, one gpsimd
    partition_all_reduce per (qb, h) replaces ones-matmuls + broadcast.
"""

import os
import sys

import numpy as np

if "/opt/trn_rl_repo" not in sys.path:
    sys.path.insert(0, "/opt/trn_rl_repo")

# ---------------------------------------------------------------- constants
B = 2
N = 2048
D = 2048
NH = 16
DH = 128
NHL = 4          # heads per core
HL = NHL * DH    # 512 local head dims
POLY = 4
EPS = 1e-6
LRPE_BASE = 10000.0

CH = 512         # projection n-chunk (columns of xT per step)
QB = 512         # query block
KB = 128         # key block
NDB = D // 128   # 16 contraction d-blocks
NCH = N // CH    # 4 chunks
NQB = N // QB    # 4 query blocks
NKBT = N // KB   # 16 key blocks total
LA = 2           # attention software-pipeline lookahead (score ahead of AV)


# ---------------------------------------------------------------- builder
def build_module(n=N):
    import concourse.bacc as bacc
    import concourse.mybir as mybir
    import concourse.tile as tile
    from concourse import bass_isa

    f32 = mybir.dt.float32
    bf16 = mybir.dt.bfloat16
    f16 = mybir.dt.float16
    AF = mybir.ActivationFunctionType

    nc = bacc.Bacc(
        "TRN2",
        target_bir_lowering=False,
        debug=False,
        enable_asserts=False,
        num_devices=8,
    )

    nch = n // CH
    nqb = n // QB

    # host-relayouted inputs (see make_in_maps)
    xt_d = nc.dram_tensor("xt", [128, nch, NDB * CH], bf16, kind="ExternalInput").ap()
    wq_d = nc.dram_tensor("wq", [128, NDB * HL], bf16, kind="ExternalInput").ap()
    wk_d = nc.dram_tensor("wk", [128, NDB * HL], bf16, kind="ExternalInput").ap()
    wv_d = nc.dram_tensor("wv", [128, NDB * HL], bf16, kind="ExternalInput").ap()
    wo_d = nc.dram_tensor("wo", [128, NHL * D], bf16, kind="ExternalInput").ap()
    cs_d = nc.dram_tensor("cs", [DH, n], f32, kind="ExternalInput").ap()
    sn_d = nc.dram_tensor("sn", [DH, n], f32, kind="ExternalInput").ap()
    mk_d = nc.dram_tensor("msk", [KB, KB], f32, kind="ExternalInput").ap()
    out_d = nc.dram_tensor("out", [n, D], f16, kind="ExternalOutput").ap()

    def mm(out, lhsT, rhs, start, stop):
        nc.tensor.matmul(out, lhsT, rhs, start=start, stop=stop)

    with tile.TileContext(nc) as tc:
        from contextlib import ExitStack

        with ExitStack() as ctx:
            persist = ctx.enter_context(tc.tile_pool(name="persist", bufs=1))
            qT = persist.tile([128, NHL * n], bf16, tag="qT", name="qT")
            kT = persist.tile([128, NHL * n], bf16, tag="kT", name="kT")
            vS = persist.tile([128, NKBT * HL], bf16, tag="vS", name="vS")
            cs_t = persist.tile([128, n], f32, tag="cs", name="cs")
            sn_t = persist.tile([128, n], f32, tag="sn", name="sn")
            nc.sync.dma_start(cs_t[:], cs_d[:, :])
            nc.sync.dma_start(sn_t[:], sn_d[:, :])

            # ------------- pass AB: q,k,v projections + rope (x loaded once)
            with tc.tile_pool(name="ab_w", bufs=1) as wpool, \
                 tc.tile_pool(name="ab_x", bufs=2) as xpool, \
                 tc.tile_pool(name="ab_t", bufs=6) as tpool, \
                 tc.tile_pool(name="ab_ps", bufs=3, space="PSUM") as pspool, \
                 tc.tile_pool(name="ab_pv", bufs=2, space="PSUM") as pvpool:
                wq_t = wpool.tile([128, NDB * HL], bf16, tag="wq", name="wq")
                wk_t = wpool.tile([128, NDB * HL], bf16, tag="wk", name="wk")
                wv_t = wpool.tile([128, NDB * HL], bf16, tag="wv", name="wv")
                # split weight loads so the first q chain can start early
                half = NDB * HL // 2
                nc.sync.dma_start(wq_t[:, 0:half], wq_d[:, 0:half])
                nc.sync.dma_start(wq_t[:, half:], wq_d[:, half:])
                nc.sync.dma_start(wk_t[:, :], wk_d[:, :])
                nc.sync.dma_start(wv_t[:, :], wv_d[:, :])

                for c in range(nch):
                    c0 = c * CH
                    xt_c = xpool.tile([128, NDB * CH], bf16, tag="xtc", name="xtc")
                    nc.sync.dma_start(xt_c[:, 0:NDB * CH // 2], xt_d[:, c, 0:NDB * CH // 2])
                    nc.sync.dma_start(xt_c[:, NDB * CH // 2:], xt_d[:, c, NDB * CH // 2:])

                    for h in range(NHL):
                        for w_t, dstT in ((wq_t, qT), (wk_t, kT)):
                            ps = pspool.tile([128, CH], f32, tag="ps", name="ps")
                            for i in range(NDB):
                                mm(ps[:], w_t[:, i * HL + h * 128: i * HL + (h + 1) * 128],
                                   xt_c[:, i * CH:(i + 1) * CH],
                                   start=(i == 0), stop=(i == NDB - 1))
                            # rope: dst = ps*CS + swap(ps)*SN
                            dst = dstT[:, h * n + c0: h * n + c0 + CH]
                            swp = tpool.tile([128, CH], f32, tag="swp", name="swp")
                            nc.scalar.copy(swp[0:64, :], ps[64:128, :])
                            nc.scalar.copy(swp[64:128, :], ps[0:64, :])
                            m1 = tpool.tile([128, CH], f32, tag="m1", name="m1")
                            nc.vector.tensor_mul(m1[:], ps[:], cs_t[:, c0:c0 + CH])
                            m2 = tpool.tile([128, CH], f32, tag="m2", name="m2")
                            nc.gpsimd.tensor_mul(m2[:], swp[:], sn_t[:, c0:c0 + CH])
                            nc.vector.tensor_add(dst, m1[:], m2[:])

                    for t2 in range(CH // 128):
                        kb = c * (CH // 128) + t2
                        psv = pvpool.tile([128, HL], f32, tag="psv", name="psv")
                        for i in range(NDB):
                            mm(psv[:], xt_c[:, i * CH + t2 * 128: i * CH + (t2 + 1) * 128],
                               wv_t[:, i * HL:(i + 1) * HL],
                               start=(i == 0), stop=(i == NDB - 1))
                        nc.scalar.copy(vS[:, kb * HL:(kb + 1) * HL], psv[:])

            # ------------- pass C: attention + Wo
            with tc.tile_pool(name="c_w", bufs=1) as wpool, \
                 tc.tile_pool(name="c_s2", bufs=3) as s2pool, \
                 tc.tile_pool(name="c_s4", bufs=4) as s4pool, \
                 tc.tile_pool(name="c_sa", bufs=2) as sapool, \
                 tc.tile_pool(name="c_rb", bufs=2) as rbpool, \
                 tc.tile_pool(name="c_on", bufs=6) as onpool, \
                 tc.tile_pool(name="c_fo", bufs=2) as fopool, \
                 tc.tile_pool(name="c_ps", bufs=3, space="PSUM") as psS, \
                 tc.tile_pool(name="c_po", bufs=2, space="PSUM") as psO, \
                 tc.tile_pool(name="c_pf", bufs=2, space="PSUM") as psF:
                wo_t = wpool.tile([128, NHL * D], bf16, tag="wo", name="wo")
                mk = wpool.tile([128, KB], f32, tag="mk", name="mk")
                nc.sync.dma_start(mk[:], mk_d[:, :])
                nc.sync.dma_start(wo_t[:, :], wo_d[:, :])

                for qb in range(nqb):
                    nkb = (qb + 1) * (QB // KB)
                    onrm = [onpool.tile([128, QB], bf16, tag=f"onrm{h}", name=f"onrm{h}")
                            for h in range(NHL)]
                    for h in range(NHL):
                        pso = psO.tile([128, QB], f32, tag="pso", name="pso")
                        sAcc = sapool.tile([128, QB], f32, tag="sacc", name="sacc")
                        s4q = {}
                        for step in range(nkb + LA):
                            if step < nkb:
                                kb = step
                                rel = kb - qb * (QB // KB)
                                cr = 0 if rel < 0 else 128 * rel
                                pss = psS.tile([128, QB], f32, tag="pss", name="pss")
                                mm(pss[:, cr:],
                                   kT[:, h * n + kb * KB: h * n + (kb + 1) * KB],
                                   qT[:, h * n + qb * QB + cr: h * n + (qb + 1) * QB],
                                   start=True, stop=True)
                                s2 = s2pool.tile([128, QB], f32, tag="s2", name="s2")
                                nc.scalar.activation(s2[:, cr:], pss[:, cr:], AF.Square)
                                if rel >= 0:
                                    nc.gpsimd.tensor_mul(s2[:, cr:cr + 128],
                                                         s2[:, cr:cr + 128], mk[:])
                                s4 = s4pool.tile([128, QB], bf16, tag="s4", name="s4")
                                nc.vector.tensor_mul(s4[:, cr:], s2[:, cr:], s2[:, cr:])
                                s4q[kb] = (s4, cr)
                            if step >= LA:
                                kb = step - LA
                                s4, cr = s4q.pop(kb)
                                mm(pso[:, cr:],
                                   vS[:, kb * HL + h * 128: kb * HL + (h + 1) * 128],
                                   s4[:, cr:],
                                   start=(kb == 0), stop=(kb == nkb - 1))
                                if kb == 0:
                                    nc.vector.tensor_copy(sAcc[:], s4[:])
                                else:
                                    nc.vector.tensor_add(sAcc[:, cr:], sAcc[:, cr:],
                                                         s4[:, cr:])
                        # denominator: cross-partition sum -> max(eps) -> 1/x
                        rbc = rbpool.tile([128, QB], f32, tag="rbc", name="rbc")
                        nc.gpsimd.partition_all_reduce(rbc[:], sAcc[:], 128,
                                                       bass_isa.ReduceOp.add)
                        rbm = rbpool.tile([128, QB], f32, tag="rbm", name="rbm")
                        nc.vector.tensor_scalar_max(rbm[:], rbc[:], EPS)
                        rbr = rbpool.tile([128, QB], f32, tag="rbr", name="rbr")
                        nc.vector.reciprocal(rbr[:], rbm[:])
                        nc.vector.tensor_mul(onrm[h][:], pso[:], rbr[:])

                    # Wo projection for this query block
                    for qt in range(QB // 128):
                        fout = fopool.tile([128, D], f16, tag="fout", name="fout")
                        for jc in range(D // 512):
                            psf = psF.tile([128, 512], f32, tag="psf", name="psf")
                            for h in range(NHL):
                                mm(psf[:], onrm[h][:, qt * 128:(qt + 1) * 128],
                                   wo_t[:, h * D + jc * 512: h * D + (jc + 1) * 512],
                                   start=(h == 0), stop=(h == NHL - 1))
                            if jc % 2 == 0:
                                nc.scalar.copy(fout[:, jc * 512:(jc + 1) * 512], psf[:])
                            else:
                                nc.vector.tensor_copy(fout[:, jc * 512:(jc + 1) * 512], psf[:])
                        r0 = qb * QB + qt * 128
                        nc.sync.dma_start(out_d[r0:r0 + 128, :], fout[:])

    nc.compile()
    return nc


# ---------------------------------------------------------------- host prep
def _rope_tables(n):
    half = DH // 2
    theta = LRPE_BASE ** (-np.arange(half, dtype=np.float64) * 2.0 / DH)
    pos = np.arange(n, dtype=np.float64)
    ang = np.outer(pos, theta)                       # [n, 64]
    cos = np.cos(ang).T.astype(np.float32)           # [64, n]
    sin = np.sin(ang).T.astype(np.float32)
    cs = np.concatenate([cos, cos], axis=0)          # [128, n]
    sn = np.concatenate([-sin, sin], axis=0)
    return np.ascontiguousarray(cs), np.ascontiguousarray(sn)


def _mask():
    kp = np.arange(KB)[:, None]
    j = np.arange(KB)[None, :]
    return (kp <= j).astype(np.float32)


def make_in_maps(x, Wq, Wk, Wv, Wo, n=N):
    import ml_dtypes
    bf16 = ml_dtypes.bfloat16

    cs, sn = _rope_tables(n)
    mk = _mask()
    nch = n // CH

    def relayout_x(xb):
        # xt [d, n] -> [128, nch, NDB*CH]: chunk c contiguous per partition
        xt = np.ascontiguousarray(xb.T)
        a = xt.reshape(NDB, 128, nch, CH).transpose(1, 2, 0, 3)
        return np.ascontiguousarray(a.reshape(128, nch, NDB * CH).astype(bf16))

    def relayout_w(Wrows):
        # W[rows,:].T [d, HL] -> [128, NDB*HL]
        w = Wrows.T.reshape(NDB, 128, HL).transpose(1, 0, 2)
        return np.ascontiguousarray(w.reshape(128, NDB * HL).astype(bf16))

    def relayout_wo(Wcols):
        # Wo[:, rows].T [HL, D] -> [128, NHL*D]
        w = Wcols.T.reshape(NHL, 128, D).transpose(1, 0, 2)
        return np.ascontiguousarray(w.reshape(128, NHL * D).astype(bf16))

    xts = [relayout_x(x[b]) for b in range(x.shape[0])]
    in_maps = []
    for core in range(8):
        b, g = core // 4, core % 4
        rows = slice(g * HL, (g + 1) * HL)
        in_maps.append({
            "xt": xts[b],
            "wq": relayout_w(Wq[rows, :]),
            "wk": relayout_w(Wk[rows, :]),
            "wv": relayout_w(Wv[rows, :]),
            "wo": relayout_wo(Wo[:, rows]),
            "cs": cs,
            "sn": sn,
            "msk": mk,
        })
    return in_maps


_NC_CACHE = {}


def _get_nc(n=N):
    if n not in _NC_CACHE:
        _NC_CACHE[n] = build_module(n)
    return _NC_CACHE[n]


def run(x, Wq, Wk, Wv, Wo, trace=False, **kw):
    from concourse.bass_utils import run_bass_kernel_spmd

    x = np.asarray(x, dtype=np.float32)
    nc = _get_nc(x.shape[1])
    in_maps = make_in_maps(x, Wq, Wk, Wv, Wo, n=x.shape[1])
    res = run_bass_kernel_spmd(nc, in_maps, core_ids=list(range(8)), trace=trace, **kw)
    outs = [np.asarray(res.results[i]["out"], dtype=np.float32) for i in range(8)]
    b0 = outs[0] + outs[1] + outs[2] + outs[3]
    b1 = outs[4] + outs[5] + outs[6] + outs[7]
    out = np.stack([b0, b1]).astype(np.float32)
    return out, res


def kernel(x, Wq, Wk, Wv, Wo):
    out, _ = run(
        np.asarray(x, np.float32),
        np.asarray(Wq, np.float32),
        np.asarray(Wk, np.float32),
        np.asarray(Wv, np.float32),
        np.asarray(Wo, np.float32),
    )
    return out


# revision 5
# speedup vs baseline: 1.6721x; 1.5326x over previous
"""Trainium2 Bass kernel for nn_PolyAttention (16-head polynomial causal attention).

Reference math (fp32):
    q = x @ Wq.T; k = x @ Wk.T; v = x @ Wv.T        (per-head dim 128, 16 heads)
    q, k = rope(q), rope(k)                          (LRPE type-1, base 10000)
    s = (q . k)^4, causal-masked, row-normalized by max(sum, 1e-6)
    out = (s @ v normalized) @ Wo.T

Sharding: 8 cores = batch(2) x head-group(4 heads each).  Each core computes its
(b, head-group) shard end-to-end plus the Wo partial projection; the host sums
the 4 partials per batch element.

v2 design (vs v1):
  - all matmul operands bf16 (host-converted); fp32 PSUM accumulate; fp16 out.
    Numerics sim: rel_fro ~7e-3 (gate 2e-2).  bf16 halves DMA + SBUF and
    enables FWL fast weight loads.
  - single merged projection pass: x loaded once, q/k/v computed per n-chunk.
  - host relayouts inputs so every chunk/weight load is ONE dma_start with
    16KB contiguous per partition line.
  - attention: scores built transposed [keys, queries]; 2-block software
    pipeline (score chain runs 2 blocks ahead of the AV chain) so the PE
    never waits on the scalar/vector square/quartic pipeline.
  - denominator off the PE: DVE accumulates s4 blocks into sAcc, one gpsimd
    partition_all_reduce per (qb, h) replaces ones-matmuls + broadcast.
"""

import os
import sys

import numpy as np

if "/opt/trn_rl_repo" not in sys.path:
    sys.path.insert(0, "/opt/trn_rl_repo")

# ---------------------------------------------------------------- constants
B = 2
N = 2048
D = 2048
NH = 16
DH = 128
NHL = 4          # heads per core
HL = NHL * DH    # 512 local head dims
POLY = 4
EPS = 1e-6
LRPE_BASE = 10000.0

CH = 512         # projection n-chunk (columns of xT per step)
QB = 512         # query block
KB = 128         # key block
NDB = D // 128   # 16 contraction d-blocks
NCH = N // CH    # 4 chunks
NQB = N // QB    # 4 query blocks
NKBT = N // KB   # 16 key blocks total
LA = 2           # attention software-pipeline lookahead (score ahead of AV)


# ---------------------------------------------------------------- builder
def build_module(n=N):
    import concourse.bacc as bacc
    import concourse.mybir as mybir
    import concourse.tile as tile
    from concourse import bass_isa

    f32 = mybir.dt.float32
    bf16 = mybir.dt.bfloat16
    f16 = mybir.dt.float16
    AF = mybir.ActivationFunctionType

    nc = bacc.Bacc(
        "TRN2",
        target_bir_lowering=False,
        debug=False,
        enable_asserts=False,
        num_devices=8,
    )

    nch = n // CH
    nqb = n // QB

    # host-relayouted inputs (see make_in_maps)
    xt_d = nc.dram_tensor("xt", [128, nch, NDB * CH], bf16, kind="ExternalInput").ap()
    wq_d = nc.dram_tensor("wq", [128, NDB * HL], bf16, kind="ExternalInput").ap()
    wk_d = nc.dram_tensor("wk", [128, NDB * HL], bf16, kind="ExternalInput").ap()
    wv_d = nc.dram_tensor("wv", [128, NDB * HL], bf16, kind="ExternalInput").ap()
    wo_d = nc.dram_tensor("wo", [128, NHL * D], bf16, kind="ExternalInput").ap()
    cs_d = nc.dram_tensor("cs", [DH, n], f32, kind="ExternalInput").ap()
    sn_d = nc.dram_tensor("sn", [DH, n], f32, kind="ExternalInput").ap()
    mk_d = nc.dram_tensor("msk", [KB, KB], bf16, kind="ExternalInput").ap()
    out_d = nc.dram_tensor("out", [n, D], f16, kind="ExternalOutput").ap()

    def mm(out, lhsT, rhs, start, stop):
        nc.tensor.matmul(out, lhsT, rhs, start=start, stop=stop)

    with tile.TileContext(nc) as tc:
        from contextlib import ExitStack

        with ExitStack() as ctx:
            persist = ctx.enter_context(tc.tile_pool(name="persist", bufs=1))
            qT = persist.tile([128, NHL * n], bf16, tag="qT", name="qT")
            kT = persist.tile([128, NHL * n], bf16, tag="kT", name="kT")
            vS = persist.tile([128, NKBT * HL], bf16, tag="vS", name="vS")
            cs_t = persist.tile([128, n], f32, tag="cs", name="cs")
            sn_t = persist.tile([128, n], f32, tag="sn", name="sn")
            nc.sync.dma_start(cs_t[:], cs_d[:, :])
            nc.sync.dma_start(sn_t[:], sn_d[:, :])

            # ------------- pass AB: q,k,v projections + rope (x loaded once)
            with tc.tile_pool(name="ab_w", bufs=1) as wpool, \
                 tc.tile_pool(name="ab_x", bufs=2) as xpool, \
                 tc.tile_pool(name="ab_t", bufs=6) as tpool, \
                 tc.tile_pool(name="ab_ps", bufs=3, space="PSUM") as pspool, \
                 tc.tile_pool(name="ab_pv", bufs=2, space="PSUM") as pvpool:
                wq_t = wpool.tile([128, NDB * HL], bf16, tag="wq", name="wq")
                wk_t = wpool.tile([128, NDB * HL], bf16, tag="wk", name="wk")
                wv_t = wpool.tile([128, NDB * HL], bf16, tag="wv", name="wv")
                # split weight loads so the first q chain can start early
                half = NDB * HL // 2
                nc.sync.dma_start(wq_t[:, 0:half], wq_d[:, 0:half])
                nc.sync.dma_start(wq_t[:, half:], wq_d[:, half:])
                nc.sync.dma_start(wk_t[:, :], wk_d[:, :])
                nc.sync.dma_start(wv_t[:, :], wv_d[:, :])

                for c in range(nch):
                    c0 = c * CH
                    xt_c = xpool.tile([128, NDB * CH], bf16, tag="xtc", name="xtc")
                    nc.sync.dma_start(xt_c[:, 0:NDB * CH // 2], xt_d[:, c, 0:NDB * CH // 2])
                    nc.sync.dma_start(xt_c[:, NDB * CH // 2:], xt_d[:, c, NDB * CH // 2:])

                    for h in range(NHL):
                        for w_t, dstT in ((wq_t, qT), (wk_t, kT)):
                            ps = pspool.tile([128, CH], f32, tag="ps", name="ps")
                            for i in range(NDB):
                                mm(ps[:], w_t[:, i * HL + h * 128: i * HL + (h + 1) * 128],
                                   xt_c[:, i * CH:(i + 1) * CH],
                                   start=(i == 0), stop=(i == NDB - 1))
                            # rope: dst = ps*CS + swap(ps)*SN
                            dst = dstT[:, h * n + c0: h * n + c0 + CH]
                            swp = tpool.tile([128, CH], f32, tag="swp", name="swp")
                            nc.scalar.copy(swp[0:64, :], ps[64:128, :])
                            nc.scalar.copy(swp[64:128, :], ps[0:64, :])
                            m1 = tpool.tile([128, CH], f32, tag="m1", name="m1")
                            nc.vector.tensor_mul(m1[:], ps[:], cs_t[:, c0:c0 + CH])
                            m2 = tpool.tile([128, CH], f32, tag="m2", name="m2")
                            nc.gpsimd.tensor_mul(m2[:], swp[:], sn_t[:, c0:c0 + CH])
                            nc.vector.tensor_add(dst, m1[:], m2[:])

                    for t2 in range(CH // 128):
                        kb = c * (CH // 128) + t2
                        psv = pvpool.tile([128, HL], f32, tag="psv", name="psv")
                        for i in range(NDB):
                            mm(psv[:], xt_c[:, i * CH + t2 * 128: i * CH + (t2 + 1) * 128],
                               wv_t[:, i * HL:(i + 1) * HL],
                               start=(i == 0), stop=(i == NDB - 1))
                        nc.scalar.copy(vS[:, kb * HL:(kb + 1) * HL], psv[:])

            # ------------- pass C: attention + Wo
            with tc.tile_pool(name="c_w", bufs=1) as wpool, \
                 tc.tile_pool(name="c_s2", bufs=3) as s2pool, \
                 tc.tile_pool(name="c_s4", bufs=4) as s4pool, \
                 tc.tile_pool(name="c_rb", bufs=2) as rbpool, \
                 tc.tile_pool(name="c_on", bufs=6) as onpool, \
                 tc.tile_pool(name="c_fo", bufs=2) as fopool, \
                 tc.tile_pool(name="c_ps", bufs=3, space="PSUM") as psS, \
                 tc.tile_pool(name="c_po", bufs=2, space="PSUM") as psO, \
                 tc.tile_pool(name="c_pd", bufs=1, space="PSUM") as psD, \
                 tc.tile_pool(name="c_pf", bufs=2, space="PSUM") as psF:
                wo_t = wpool.tile([128, NHL * D], bf16, tag="wo", name="wo")
                mk = wpool.tile([128, KB], bf16, tag="mk", name="mk")
                ones = wpool.tile([128, 1], bf16, tag="ones", name="ones")
                epsv = wpool.tile([1, 1], bf16, tag="epsv", name="epsv")
                oner = wpool.tile([1, QB], bf16, tag="oner", name="oner")
                nc.vector.memset(ones[:], 1.0)
                nc.vector.memset(epsv[:], EPS)
                nc.vector.memset(oner[:], 1.0)
                nc.sync.dma_start(mk[:], mk_d[:, :])
                nc.sync.dma_start(wo_t[:, :], wo_d[:, :])

                for qb in range(nqb):
                    nkb = (qb + 1) * (QB // KB)
                    onrm = [onpool.tile([128, QB], bf16, tag=f"onrm{h}", name=f"onrm{h}")
                            for h in range(NHL)]
                    pend = []       # deferred normalize-mul emission
                    for h in range(NHL):
                        pso = psO.tile([128, QB], f32, tag="pso", name="pso")
                        psd = psD.tile([1, QB], f32, tag="psd", name="psd")
                        s4q = {}
                        for step in range(nkb + LA):
                            if step == 1 and pend:
                                pend.pop()()
                            if step < nkb:
                                kb = step
                                rel = kb - qb * (QB // KB)
                                cr = 0 if rel < 0 else 128 * rel
                                pss = psS.tile([128, QB], f32, tag="pss", name="pss")
                                mm(pss[:, cr:],
                                   kT[:, h * n + kb * KB: h * n + (kb + 1) * KB],
                                   qT[:, h * n + qb * QB + cr: h * n + (qb + 1) * QB],
                                   start=True, stop=True)
                                s2 = s2pool.tile([128, QB], bf16, tag="s2", name="s2")
                                nc.scalar.activation(s2[:, cr:], pss[:, cr:], AF.Square)
                                if rel >= 0:
                                    nc.vector.tensor_mul(s2[:, cr:cr + 128],
                                                         s2[:, cr:cr + 128], mk[:])
                                s4 = s4pool.tile([128, QB], bf16, tag="s4", name="s4")
                                nc.vector.tensor_mul(s4[:, cr:], s2[:, cr:], s2[:, cr:])
                                s4q[kb] = (s4, cr)
                            if step >= LA:
                                kb = step - LA
                                s4, cr = s4q.pop(kb)
                                mm(pso[:, cr:],
                                   vS[:, kb * HL + h * 128: kb * HL + (h + 1) * 128],
                                   s4[:, cr:],
                                   start=(kb == 0), stop=(kb == nkb - 1))
                                mm(psd[0:1, cr:], ones[:, 0:1], s4[:, cr:],
                                   start=(kb == 0), stop=False)
                        # + eps, so the reciprocal input is strictly positive
                        # (row-0 denominators are >=7e-3 here, so +eps == max(,eps))
                        mm(psd[0:1, :], epsv[0:1, 0:1], oner[0:1, :],
                           start=False, stop=True)
                        rbr = rbpool.tile([1, QB], f32, tag="rbr", name="rbr")
                        nc.vector.reciprocal_approx_fast(rbr[:], psd[0:1, :])
                        rbc = rbpool.tile([128, QB], f32, tag="rbc", name="rbc")
                        nc.gpsimd.partition_broadcast(rbc[:], rbr[:])

                        def _norm(h=h, pso=pso, rbc=rbc):
                            nc.vector.tensor_mul(onrm[h][:], pso[:], rbc[:])
                        pend.append(_norm)
                    while pend:
                        pend.pop()()

                    # Wo projection for this query block
                    for qt in range(QB // 128):
                        fout = fopool.tile([128, D], f16, tag="fout", name="fout")
                        for jc in range(D // 512):
                            psf = psF.tile([128, 512], f32, tag="psf", name="psf")
                            for h in range(NHL):
                                mm(psf[:], onrm[h][:, qt * 128:(qt + 1) * 128],
                                   wo_t[:, h * D + jc * 512: h * D + (jc + 1) * 512],
                                   start=(h == 0), stop=(h == NHL - 1))
                            if jc % 2 == 0:
                                nc.scalar.copy(fout[:, jc * 512:(jc + 1) * 512], psf[:])
                            else:
                                nc.vector.tensor_copy(fout[:, jc * 512:(jc + 1) * 512], psf[:])
                        r0 = qb * QB + qt * 128
                        nc.sync.dma_start(out_d[r0:r0 + 128, :], fout[:])

    nc.compile()
    return nc


# ---------------------------------------------------------------- host prep
def _rope_tables(n):
    half = DH // 2
    theta = LRPE_BASE ** (-np.arange(half, dtype=np.float64) * 2.0 / DH)
    pos = np.arange(n, dtype=np.float64)
    ang = np.outer(pos, theta)                       # [n, 64]
    cos = np.cos(ang).T.astype(np.float32)           # [64, n]
    sin = np.sin(ang).T.astype(np.float32)
    cs = np.concatenate([cos, cos], axis=0)          # [128, n]
    sn = np.concatenate([-sin, sin], axis=0)
    return np.ascontiguousarray(cs), np.ascontiguousarray(sn)


def _mask():
    kp = np.arange(KB)[:, None]
    j = np.arange(KB)[None, :]
    return (kp <= j).astype(np.float32)


def make_in_maps(x, Wq, Wk, Wv, Wo, n=N):
    import ml_dtypes
    bf16 = ml_dtypes.bfloat16

    cs, sn = _rope_tables(n)
    mk = _mask()
    nch = n // CH

    def relayout_x(xb):
        # xt [d, n] -> [128, nch, NDB*CH]: chunk c contiguous per partition
        xt = np.ascontiguousarray(xb.T)
        a = xt.reshape(NDB, 128, nch, CH).transpose(1, 2, 0, 3)
        return np.ascontiguousarray(a.reshape(128, nch, NDB * CH).astype(bf16))

    def relayout_w(Wrows):
        # W[rows,:].T [d, HL] -> [128, NDB*HL]
        w = Wrows.T.reshape(NDB, 128, HL).transpose(1, 0, 2)
        return np.ascontiguousarray(w.reshape(128, NDB * HL).astype(bf16))

    def relayout_wo(Wcols):
        # Wo[:, rows].T [HL, D] -> [128, NHL*D]
        w = Wcols.T.reshape(NHL, 128, D).transpose(1, 0, 2)
        return np.ascontiguousarray(w.reshape(128, NHL * D).astype(bf16))

    xts = [relayout_x(x[b]) for b in range(x.shape[0])]
    in_maps = []
    for core in range(8):
        b, g = core // 4, core % 4
        rows = slice(g * HL, (g + 1) * HL)
        in_maps.append({
            "xt": xts[b],
            "wq": relayout_w(Wq[rows, :]),
            "wk": relayout_w(Wk[rows, :]),
            "wv": relayout_w(Wv[rows, :]),
            "wo": relayout_wo(Wo[:, rows]),
            "cs": cs,
            "sn": sn,
            "msk": mk.astype(bf16),
        })
    return in_maps


_NC_CACHE = {}


def _get_nc(n=N):
    if n not in _NC_CACHE:
        _NC_CACHE[n] = build_module(n)
    return _NC_CACHE[n]


def run(x, Wq, Wk, Wv, Wo, trace=False, **kw):
    from concourse.bass_utils import run_bass_kernel_spmd

    x = np.asarray(x, dtype=np.float32)
    nc = _get_nc(x.shape[1])
    in_maps = make_in_maps(x, Wq, Wk, Wv, Wo, n=x.shape[1])
    res = run_bass_kernel_spmd(nc, in_maps, core_ids=list(range(8)), trace=trace, **kw)
    outs = [np.asarray(res.results[i]["out"], dtype=np.float32) for i in range(8)]
    b0 = outs[0] + outs[1] + outs[2] + outs[3]
    b1 = outs[4] + outs[5] + outs[6] + outs[7]
    out = np.stack([b0, b1]).astype(np.float32)
    return out, res


def kernel(x, Wq, Wk, Wv, Wo):
    out, _ = run(
        np.asarray(x, np.float32),
        np.asarray(Wq, np.float32),
        np.asarray(Wk, np.float32),
        np.asarray(Wv, np.float32),
        np.asarray(Wo, np.float32),
    )
    return out


# revision 9
# speedup vs baseline: 1.7238x; 1.0310x over previous
"""Trainium2 Bass kernel for nn_PolyAttention (16-head polynomial causal attention).

Reference math (fp32):
    q = x @ Wq.T; k = x @ Wk.T; v = x @ Wv.T        (per-head dim 128, 16 heads)
    q, k = rope(q), rope(k)                          (LRPE type-1, base 10000)
    s = (q . k)^4, causal-masked, row-normalized by max(sum, 1e-6)
    out = (s @ v normalized) @ Wo.T

Sharding: 8 cores = batch(2) x head-group(4 heads each).  Each core computes its
(b, head-group) shard end-to-end plus the Wo partial projection; the host sums
the 4 partials per batch element.

v2 design (vs v1):
  - all matmul operands bf16 (host-converted); fp32 PSUM accumulate; fp16 out.
    Numerics sim: rel_fro ~7e-3 (gate 2e-2).  bf16 halves DMA + SBUF and
    enables FWL fast weight loads.
  - single merged projection pass: x loaded once, q/k/v computed per n-chunk.
  - host relayouts inputs so every chunk/weight load is ONE dma_start with
    16KB contiguous per partition line.
  - attention: scores built transposed [keys, queries]; 2-block software
    pipeline (score chain runs 2 blocks ahead of the AV chain) so the PE
    never waits on the scalar/vector square/quartic pipeline.
  - denominator off the PE: DVE accumulates s4 blocks into sAcc, one gpsimd
    partition_all_reduce per (qb, h) replaces ones-matmuls + broadcast.
"""

import os
import sys

import numpy as np

if "/opt/trn_rl_repo" not in sys.path:
    sys.path.insert(0, "/opt/trn_rl_repo")

# ---------------------------------------------------------------- constants
B = 2
N = 2048
D = 2048
NH = 16
DH = 128
NHL = 4          # heads per core
HL = NHL * DH    # 512 local head dims
POLY = 4
EPS = 1e-6
LRPE_BASE = 10000.0

CH = 512         # projection n-chunk (columns of xT per step)
QB = 512         # query block
KB = 128         # key block
NDB = D // 128   # 16 contraction d-blocks
NCH = N // CH    # 4 chunks
NQB = N // QB    # 4 query blocks
NKBT = N // KB   # 16 key blocks total
LA = 2           # attention software-pipeline lookahead (score ahead of AV)


# ---------------------------------------------------------------- builder
def build_module(n=N):
    import concourse.bacc as bacc
    import concourse.mybir as mybir
    import concourse.tile as tile
    from concourse import bass_isa

    f32 = mybir.dt.float32
    bf16 = mybir.dt.bfloat16
    f16 = mybir.dt.float16
    AF = mybir.ActivationFunctionType

    nc = bacc.Bacc(
        "TRN2",
        target_bir_lowering=False,
        debug=False,
        enable_asserts=False,
        num_devices=8,
    )

    nch = n // CH
    nqb = n // QB

    # host-relayouted inputs (see make_in_maps)
    xt_d = nc.dram_tensor("xt", [128, nch, NDB * CH], bf16, kind="ExternalInput").ap()
    wq_d = nc.dram_tensor("wq", [128, NDB * HL], bf16, kind="ExternalInput").ap()
    wk_d = nc.dram_tensor("wk", [128, NDB * HL], bf16, kind="ExternalInput").ap()
    wv_d = nc.dram_tensor("wv", [128, NDB * HL], bf16, kind="ExternalInput").ap()
    wo_d = nc.dram_tensor("wo", [128, NHL * D], bf16, kind="ExternalInput").ap()
    cs_d = nc.dram_tensor("cs", [DH, n], f32, kind="ExternalInput").ap()
    sn_d = nc.dram_tensor("sn", [DH, n], f32, kind="ExternalInput").ap()
    mk_d = nc.dram_tensor("msk", [KB, KB], bf16, kind="ExternalInput").ap()
    out_d = nc.dram_tensor("out", [n, D], f16, kind="ExternalOutput").ap()

    def mm(out, lhsT, rhs, start, stop):
        nc.tensor.matmul(out, lhsT, rhs, start=start, stop=stop)

    with tile.TileContext(nc) as tc:
        from contextlib import ExitStack

        with ExitStack() as ctx:
            persist = ctx.enter_context(tc.tile_pool(name="persist", bufs=1))
            qT = persist.tile([128, NHL * n], bf16, tag="qT", name="qT")
            kT = persist.tile([128, NHL * n], bf16, tag="kT", name="kT")
            vS = persist.tile([128, NKBT * HL], bf16, tag="vS", name="vS")
            # per-half rope tables (separate tiles => fine-grained DMA deps)
            nh2 = n // 2
            cs_t = [persist.tile([128, nh2], f32, tag=f"cs{i}", name=f"cs{i}") for i in range(2)]
            sn_t = [persist.tile([128, nh2], f32, tag=f"sn{i}", name=f"sn{i}") for i in range(2)]

            # PSUM bank plan: AB pools (3 banks) are created first; pass C's
            # psS/psO/psD land on the remaining 5 banks so the first attention
            # matmuls never wait on AB's last PSUM readers.  psF is created
            # after the AB pools close and reuses their banks (Wo starts late).
            shps = ctx.enter_context(tc.tile_pool(name="shps", bufs=3, space="PSUM"))
            psS = ctx.enter_context(tc.tile_pool(name="c_ps", bufs=2, space="PSUM"))
            psO = ctx.enter_context(tc.tile_pool(name="c_po", bufs=2, space="PSUM"))
            psD = ctx.enter_context(tc.tile_pool(name="c_pd", bufs=1, space="PSUM"))

            # ------------- pass AB: q,k,v projections + rope (x loaded once)
            with tc.tile_pool(name="ab_w", bufs=1) as wpool, \
                 tc.tile_pool(name="ab_x", bufs=2) as xpool, \
                 tc.tile_pool(name="ab_t", bufs=6) as tpool:
                wq_t = wpool.tile([128, NDB * HL], bf16, tag="wq", name="wq")
                wk_t = wpool.tile([128, NDB * HL], bf16, tag="wk", name="wk")
                wv_t = wpool.tile([128, NDB * HL], bf16, tag="wv", name="wv")
                xt_cs = []
                for c in range(nch):
                    xt_cs.append(xpool.tile([128, NDB * CH], bf16, tag="xtc", name="xtc"))
                # startup ordering: interleave wq and xt0 quarters so the first
                # q chain starts after ~1MB, then chunk-0 rope tables, then the
                # rest in first-use order.
                qtr = NDB * HL // 4
                for p in range(4):
                    nc.sync.dma_start(wq_t[:, p * qtr:(p + 1) * qtr], wq_d[:, p * qtr:(p + 1) * qtr])
                    nc.sync.dma_start(xt_cs[0][:, p * qtr:(p + 1) * qtr], xt_d[:, 0, p * qtr:(p + 1) * qtr])
                nc.sync.dma_start(cs_t[0][:, 0:CH], cs_d[:, 0:CH])
                nc.sync.dma_start(sn_t[0][:, 0:CH], sn_d[:, 0:CH])
                half = NDB * HL // 2
                for p in range(2):
                    nc.sync.dma_start(wk_t[:, p * half:(p + 1) * half], wk_d[:, p * half:(p + 1) * half])
                nc.sync.dma_start(cs_t[0][:, CH:], cs_d[:, CH:nh2])
                nc.sync.dma_start(sn_t[0][:, CH:], sn_d[:, CH:nh2])
                for p in range(2):
                    nc.sync.dma_start(wv_t[:, p * half:(p + 1) * half], wv_d[:, p * half:(p + 1) * half])
                nc.sync.dma_start(cs_t[1][:], cs_d[:, nh2:])
                nc.sync.dma_start(sn_t[1][:], sn_d[:, nh2:])

                for c in range(nch):
                    c0 = c * CH
                    xt_c = xt_cs[c]
                    if c > 0:
                        nc.sync.dma_start(xt_c[:, 0:half], xt_d[:, c, 0:half])
                        nc.sync.dma_start(xt_c[:, half:], xt_d[:, c, half:])
                    csh = cs_t[(c0 // nh2)][:, c0 % nh2: c0 % nh2 + CH]
                    snh = sn_t[(c0 // nh2)][:, c0 % nh2: c0 % nh2 + CH]

                    for w_t, dstT in ((wq_t, qT), (wk_t, kT)):
                        for h in range(NHL):
                            ps = shps.tile([128, CH], f32, tag="ps", name="ps")
                            for i in range(NDB):
                                mm(ps[:], w_t[:, i * HL + h * 128: i * HL + (h + 1) * 128],
                                   xt_c[:, i * CH:(i + 1) * CH],
                                   start=(i == 0), stop=(i == NDB - 1))
                            # rope: dst = ps*CS + swap(ps)*SN
                            dst = dstT[:, h * n + c0: h * n + c0 + CH]
                            swp = tpool.tile([128, CH], f32, tag="swp", name="swp")
                            nc.scalar.copy(swp[0:64, :], ps[64:128, :])
                            nc.scalar.copy(swp[64:128, :], ps[0:64, :])
                            m1 = tpool.tile([128, CH], f32, tag="m1", name="m1")
                            nc.vector.tensor_mul(m1[:], ps[:], csh)
                            m2 = tpool.tile([128, CH], f32, tag="m2", name="m2")
                            nc.gpsimd.tensor_mul(m2[:], swp[:], snh)
                            nc.vector.tensor_add(dst, m1[:], m2[:])

                    for t2 in range(CH // 128):
                        kb = c * (CH // 128) + t2
                        psv = shps.tile([128, HL], f32, tag="ps", name="psv")
                        for i in range(NDB):
                            mm(psv[:], xt_c[:, i * CH + t2 * 128: i * CH + (t2 + 1) * 128],
                               wv_t[:, i * HL:(i + 1) * HL],
                               start=(i == 0), stop=(i == NDB - 1))
                        nc.scalar.copy(vS[:, kb * HL:(kb + 1) * HL], psv[:])

            # ------------- pass C: attention + Wo
            with tc.tile_pool(name="c_w", bufs=1) as wpool, \
                 tc.tile_pool(name="c_s2", bufs=3) as s2pool, \
                 tc.tile_pool(name="c_s4", bufs=4) as s4pool, \
                 tc.tile_pool(name="c_rb", bufs=2) as rbpool, \
                 tc.tile_pool(name="c_on", bufs=6) as onpool, \
                 tc.tile_pool(name="c_fo", bufs=2) as fopool:
                wo_t = wpool.tile([128, NHL * D], bf16, tag="wo", name="wo")
                mk = wpool.tile([128, KB], bf16, tag="mk", name="mk")
                ones = wpool.tile([128, 1], bf16, tag="ones", name="ones")
                epsv = wpool.tile([1, 1], bf16, tag="epsv", name="epsv")
                oner = wpool.tile([1, QB], bf16, tag="oner", name="oner")
                nc.vector.memset(ones[:], 1.0)
                nc.vector.memset(epsv[:], EPS)
                nc.vector.memset(oner[:], 1.0)
                nc.sync.dma_start(mk[:], mk_d[:, :])
                nc.sync.dma_start(wo_t[:, :], wo_d[:, :])

                for qb in range(nqb):
                    nkb = (qb + 1) * (QB // KB)
                    onrm = [onpool.tile([128, QB], bf16, tag=f"onrm{h}", name=f"onrm{h}")
                            for h in range(NHL)]
                    pend = []       # deferred normalize-mul emission
                    for h in range(NHL):
                        pso = psO.tile([128, QB], f32, tag="pso", name="pso")
                        psd = psD.tile([1, QB], f32, tag="psd", name="psd")
                        s4q = {}
                        for step in range(nkb + LA):
                            if step == 1 and pend:
                                pend.pop()()
                            if step < nkb:
                                kb = step
                                rel = kb - qb * (QB // KB)
                                cr = 0 if rel < 0 else 128 * rel
                                pss = psS.tile([128, QB], f32, tag="pss", name="pss")
                                mm(pss[:, cr:],
                                   kT[:, h * n + kb * KB: h * n + (kb + 1) * KB],
                                   qT[:, h * n + qb * QB + cr: h * n + (qb + 1) * QB],
                                   start=True, stop=True)
                                s2 = s2pool.tile([128, QB], bf16, tag="s2", name="s2")
                                nc.scalar.activation(s2[:, cr:], pss[:, cr:], AF.Square)
                                if rel >= 0:
                                    nc.vector.tensor_mul(s2[:, cr:cr + 128],
                                                         s2[:, cr:cr + 128], mk[:])
                                s4 = s4pool.tile([128, QB], bf16, tag="s4", name="s4")
                                nc.vector.tensor_mul(s4[:, cr:], s2[:, cr:], s2[:, cr:])
                                s4q[kb] = (s4, cr)
                            if step >= LA:
                                kb = step - LA
                                s4, cr = s4q.pop(kb)
                                mm(pso[:, cr:],
                                   vS[:, kb * HL + h * 128: kb * HL + (h + 1) * 128],
                                   s4[:, cr:],
                                   start=(kb == 0), stop=(kb == nkb - 1))
                                mm(psd[0:1, cr:], ones[:, 0:1], s4[:, cr:],
                                   start=(kb == 0), stop=False)
                        # + eps, so the reciprocal input is strictly positive
                        # (row-0 denominators are >=7e-3 here, so +eps == max(,eps))
                        mm(psd[0:1, :], epsv[0:1, 0:1], oner[0:1, :],
                           start=False, stop=True)
                        rbr = rbpool.tile([1, QB], f32, tag="rbr", name="rbr")
                        nc.vector.reciprocal_approx_fast(rbr[:], psd[0:1, :])
                        rbc = rbpool.tile([128, QB], f32, tag="rbc", name="rbc")
                        nc.gpsimd.partition_broadcast(rbc[:], rbr[:])

                        def _norm(h=h, pso=pso, rbc=rbc):
                            nc.vector.tensor_mul(onrm[h][:], pso[:], rbc[:])
                        pend.append(_norm)
                    while pend:
                        pend.pop()()

                    # Wo projection for this query block.  h0-2 partials are
                    # emitted for a pair of jc chains before their h3 matmuls,
                    # giving the PE ~6 matmuls of backlog to hide the last
                    # head's normalize-chain latency.
                    for qt in range(QB // 128):
                        fout = fopool.tile([128, D], f16, tag="fout", name="fout")
                        for jp in range(D // 1024):
                            psfs = []
                            for jc in (2 * jp, 2 * jp + 1):
                                psf = shps.tile([128, 512], f32, tag="ps", name="psf")
                                for h in range(NHL - 1):
                                    mm(psf[:], onrm[h][:, qt * 128:(qt + 1) * 128],
                                       wo_t[:, h * D + jc * 512: h * D + (jc + 1) * 512],
                                       start=(h == 0), stop=False)
                                psfs.append((jc, psf))
                            for jc, psf in psfs:
                                mm(psf[:], onrm[NHL - 1][:, qt * 128:(qt + 1) * 128],
                                   wo_t[:, (NHL - 1) * D + jc * 512: (NHL - 1) * D + (jc + 1) * 512],
                                   start=False, stop=True)
                                if jc % 2 == 0:
                                    nc.scalar.copy(fout[:, jc * 512:(jc + 1) * 512], psf[:])
                                else:
                                    nc.vector.tensor_copy(fout[:, jc * 512:(jc + 1) * 512], psf[:])
                        r0 = qb * QB + qt * 128
                        nc.sync.dma_start(out_d[r0:r0 + 128, :], fout[:])

    nc.compile()
    return nc


# ---------------------------------------------------------------- host prep
def _rope_tables(n):
    half = DH // 2
    theta = LRPE_BASE ** (-np.arange(half, dtype=np.float64) * 2.0 / DH)
    pos = np.arange(n, dtype=np.float64)
    ang = np.outer(pos, theta)                       # [n, 64]
    cos = np.cos(ang).T.astype(np.float32)           # [64, n]
    sin = np.sin(ang).T.astype(np.float32)
    cs = np.concatenate([cos, cos], axis=0)          # [128, n]
    sn = np.concatenate([-sin, sin], axis=0)
    return np.ascontiguousarray(cs), np.ascontiguousarray(sn)


def _mask():
    kp = np.arange(KB)[:, None]
    j = np.arange(KB)[None, :]
    return (kp <= j).astype(np.float32)


def make_in_maps(x, Wq, Wk, Wv, Wo, n=N):
    import ml_dtypes
    bf16 = ml_dtypes.bfloat16

    cs, sn = _rope_tables(n)
    mk = _mask()
    nch = n // CH

    def relayout_x(xb):
        # xt [d, n] -> [128, nch, NDB*CH]: chunk c contiguous per partition
        xt = np.ascontiguousarray(xb.T)
        a = xt.reshape(NDB, 128, nch, CH).transpose(1, 2, 0, 3)
        return np.ascontiguousarray(a.reshape(128, nch, NDB * CH).astype(bf16))

    def relayout_w(Wrows):
        # W[rows,:].T [d, HL] -> [128, NDB*HL]
        w = Wrows.T.reshape(NDB, 128, HL).transpose(1, 0, 2)
        return np.ascontiguousarray(w.reshape(128, NDB * HL).astype(bf16))

    def relayout_wo(Wcols):
        # Wo[:, rows].T [HL, D] -> [128, NHL*D]
        w = Wcols.T.reshape(NHL, 128, D).transpose(1, 0, 2)
        return np.ascontiguousarray(w.reshape(128, NHL * D).astype(bf16))

    xts = [relayout_x(x[b]) for b in range(x.shape[0])]
    in_maps = []
    for core in range(8):
        b, g = core // 4, core % 4
        rows = slice(g * HL, (g + 1) * HL)
        in_maps.append({
            "xt": xts[b],
            "wq": relayout_w(Wq[rows, :]),
            "wk": relayout_w(Wk[rows, :]),
            "wv": relayout_w(Wv[rows, :]),
            "wo": relayout_wo(Wo[:, rows]),
            "cs": cs,
            "sn": sn,
            "msk": mk.astype(bf16),
        })
    return in_maps


_NC_CACHE = {}


def _get_nc(n=N):
    if n not in _NC_CACHE:
        _NC_CACHE[n] = build_module(n)
    return _NC_CACHE[n]


def run(x, Wq, Wk, Wv, Wo, trace=False, **kw):
    from concourse.bass_utils import run_bass_kernel_spmd

    x = np.asarray(x, dtype=np.float32)
    nc = _get_nc(x.shape[1])
    in_maps = make_in_maps(x, Wq, Wk, Wv, Wo, n=x.shape[1])
    res = run_bass_kernel_spmd(nc, in_maps, core_ids=list(range(8)), trace=trace, **kw)
    outs = [np.asarray(res.results[i]["out"], dtype=np.float32) for i in range(8)]
    b0 = outs[0] + outs[1] + outs[2] + outs[3]
    b1 = outs[4] + outs[5] + outs[6] + outs[7]
    out = np.stack([b0, b1]).astype(np.float32)
    return out, res


def kernel(x, Wq, Wk, Wv, Wo):
    out, _ = run(
        np.asarray(x, np.float32),
        np.asarray(Wq, np.float32),
        np.asarray(Wk, np.float32),
        np.asarray(Wv, np.float32),
        np.asarray(Wo, np.float32),
    )
    return out
